# revision 19
# baseline (speedup 1.0000x reference)
"""DSH loss kernel for Trainium2 (8 NeuronCores, Bass/Tile).

Math (reference):
    U[ind] = u; Y[ind] = y
    raw[b,n]  = ||u_b||^2 - 2 u_b.U_n + ||U_n||^2          (>= 0 mathematically)
    dist      = max(raw, 0)
    match[b,n]= y_b . Y_n          (integer >= 0)
    m         = (match == 0)       ("mismatch" mask, statistically ~never 1)
    loss1 = mean( (1-m)*0.5*dist + m*0.5*relu(M - dist) )
    loss2 = ALPHA * mean(|1 - sign(u)|)

Decomposition (v5):
    2*B*N*loss1 = S_raw + sum_{m=1} [ relu(M - raw) - raw ]
      S_raw factorizes (N*sum(u_sq) + B*sum(U_sq) - 2*colsum(u).colsum(U))
      -> exact fp64 on host. Distances never touch the device: the
      correction only needs the LOCATIONS of match==0 pairs, found from
      the labels alone.

Device work per core (shard = 12500 gallery rows, padded to 12800):
    match[b,n] = y_b . Y_n as fp8e4 DoubleRow matmuls (exact 0/1
    operands, fp32 PSUM accumulate => exact integer counts):
      - stationary = batch labels y (4 tiles of 128 -> only 4 weight
        loads for the whole kernel), moving = gallery label stream.
      - DoubleRow packs the 100-class contraction as [64, 2] (classes
        c = p + 64j), halving PE row time.
      - a short warm-up burst of dummy matmuls (y x y) runs during the
        initial gallery DMA so the PE leaves its cold p-state before
        the real stream starts.
    Zero-match detector, one probe per pair, split three ways per
    triple of [128,1024] PSUM tiles (partition = batch item, free =
    gallery column):
      - tile 0 is DMA-mirrored PSUM->SBUF (two half-tile copies on the
        scalar + vector rings, which are otherwise idle),
      - tile 1 + mirror feed ONE VectorE tensor_tensor_reduce
        (accum = min(min(a,b)), < 0.5 flags a zero-match) -- the DVE
        streams both operands concurrently, so two tiles cost one pass,
      - tile 2 goes to ScalarE activation Relu(0.5 - s) with accum_out
        (> 0.25 flags a zero-match).
    Narrow tail tiles use a direct DVE min-reduce. Flagged
    (batch, gallery-span) candidates are re-checked exactly on host in
    fp64 (normally there are none).
"""

import numpy as np
import ml_dtypes

import concourse.bass as bass
import concourse.mybir as mybir
import concourse.tile as tile
from concourse import bacc
from concourse.bass_utils import run_bass_kernel_spmd

# Problem constants (hardcoded per harness contract)
B = 512
BIT = 64
C = 100
N = 100000
N_CORES = 8
N_SH = N // N_CORES          # 12500
M_MARGIN = 2.0 * BIT         # 128.0
ALPHA = 0.1
CH = 512                     # gallery cols per matmul chunk (1 PSUM bank)
N_BT = B // 128              # batch tiles (stationary)
N_WARMUP = 8                 # PE warm-up dummy matmuls

BF16 = ml_dtypes.bfloat16
FP8 = ml_dtypes.float8_e4m3


def _layout(n_sh: int):
    nch = -(-n_sh // CH)          # gallery chunks per batch tile
    n_pad = nch * CH
    nwide = nch // 2              # [128,1024] units
    narrow = nch % 2              # trailing [128,512] unit
    return n_pad, nch, nwide, narrow


def _schedule(n_sh: int):
    """Deterministic EW op list shared by build / decode / model.

    Wide [128,1024] units pair up: the first is ENC'd by ScalarE
    (relu(s-0.5) -> SBUF bf16, no accumulator), then one VectorE
    tensor_tensor_reduce min-reduces the pair (PSUM direct + encoded
    SBUF) into one accD column. Narrow tail units (and a leftover
    unpaired wide) use a ScalarE relu(0.5-s) accumulate column.

    Returns (ops, n_cols). ops = (kind, col, units); kind 'TTR' covers
    [enc_unit, direct_unit] (flag < 0.25), kind 'ACT' covers one unit
    (flag > 0.25).
    """
    n_pad, nch, nwide, narrow = _layout(n_sh)
    units = []
    for bt in range(N_BT):
        for k in range(nwide):
            units.append((bt, 2 * k * CH, 2 * (k + 1) * CH))
        if narrow:
            units.append((bt, 2 * nwide * CH, nch * CH))
    # greedy balance by measured per-instr cost (ACT 1452/799,
    # DVE 1221/658 ns for wide/narrow units)
    ops = []
    t_act = t_dve = 0.0
    for col, unit in enumerate(units):
        wide = unit[2] - unit[1] > CH
        ca, cd = (1452.0, 1221.0) if wide else (799.0, 658.0)
        if t_act + ca <= t_dve + cd:
            ops.append(("ACT", col, [unit]))
            t_act += ca
        else:
            ops.append(("DVE", col, [unit]))
            t_dve += cd
    return ops, len(units)


def _build_program(n_sh: int):
    fp32 = mybir.dt.float32
    bf16 = mybir.dt.bfloat16
    fp8 = mybir.dt.float8e4
    nc = bacc.Bacc("TRN2", target_bir_lowering=False)

    n_pad, nch, nwide, narrow = _layout(n_sh)
    ops, n_cols = _schedule(n_sh)
    # index ops by their trigger unit (bt, k-range start)
    DR = mybir.MatmulPerfMode.DoubleRow
    amin = mybir.AluOpType.min

    y_d = nc.declare_dram_parameter("y2", [64, 2 * B], fp8, isOutput=False)
    YT_d = nc.declare_dram_parameter("YT", [64, 2 * n_sh], fp8, isOutput=False)
    accD_d = nc.declare_dram_parameter("accD", [128, n_cols], fp32, isOutput=True)

    with tile.TileContext(nc) as tc:
        with (
            tc.tile_pool(name="resident", bufs=1) as resident,
            tc.tile_pool(name="scr", bufs=2) as scrp,
            tc.tile_pool(name="psum", bufs=4, space="PSUM") as psump,
        ):
            y_sb = resident.tile([64, 2, B], fp8, tag="y")
            YT_sb = resident.tile([64, 2, n_pad], fp8, tag="YT")
            accD = resident.tile([128, n_cols], fp32, tag="accD")
            bias_h = resident.tile([128, 1], fp32, tag="biash")
            bias_m = resident.tile([128, 1], fp32, tag="biasm")

            # stationary labels first (tiny; gates warm-up + every MM)
            nc.scalar.dma_start(y_sb[:, 0, :], y_d[:, :B])
            nc.scalar.dma_start(y_sb[:, 1, :], y_d[:, B:])
            # gallery stream: plane j=0 on sync, j=1 on gpsimd, small first
            for j, q in ((0, nc.sync), (1, nc.gpsimd)):
                s = 0
                widths = [512, 512, 1024, 2048, 4096] + [8192] * 8
                for w in widths:
                    if s >= n_sh:
                        break
                    w = min(w, n_sh - s)
                    q.dma_start(YT_sb[:, j, s : s + w],
                                YT_d[:, j * n_sh + s : j * n_sh + s + w])
                    s += w
                if s < n_sh:
                    q.dma_start(YT_sb[:, j, s:], YT_d[:, j * n_sh + s :])
            if n_pad > n_sh:
                nc.vector.memset(YT_sb[:, 0, n_sh:], 1.0)
                nc.vector.memset(YT_sb[:, 1, n_sh:], 1.0)
            nc.vector.memset(bias_h[:], 0.5)
            nc.vector.memset(bias_m[:], -0.5)
            nc.vector.memset(accD[:], 1.0)

            role = {}       # unit -> (kind, col)
            for kind, col, units in ops:
                role[units[0]] = (kind, col)

            first = True
            for bt in range(N_BT):
                stat = y_sb[:, :, bt * 128 : (bt + 1) * 128]
                for k in range(nwide + narrow):
                    wide = k < nwide
                    g0 = 2 * k * CH
                    g1 = g0 + (2 * CH if wide else CH)
                    unit = (bt, g0, g1)
                    x = psump.tile([128, 1024], fp32, tag="x")
                    if first:
                        # PE warm-up: dummy y x y matmuls during DMA wait
                        for _ in range(N_WARMUP):
                            nc.tensor.matmul(
                                x[:, :512], lhsT=stat, rhs=y_sb[:, :, :],
                                start=True, stop=True, perf_mode=DR,
                            )
                        first = False
                    for c0 in range(g0, g1, CH):
                        nc.tensor.matmul(
                            x[:, c0 - g0 : c0 - g0 + CH],
                            lhsT=stat,
                            rhs=YT_sb[:, :, c0 : c0 + CH],
                            start=True, stop=True, perf_mode=DR,
                        )
                    xa = x[:, : g1 - g0]

                    kind, col = role[unit]
                    ca = accD[:, col : col + 1]
                    if kind == "ACT":
                        scrA = scrp.tile([128, 1024], bf16, tag="scrA")
                        nc.scalar.activation(
                            scrA[:, : g1 - g0], xa,
                            mybir.ActivationFunctionType.Relu,
                            bias=bias_h[:], scale=-1.0,
                            accum_out=ca,
                        )
                    else:
                        nc.vector.tensor_reduce(
                            ca, xa, mybir.AxisListType.X, amin,
                        )

            cut = max(0, n_cols - 5)
            if cut:
                nc.sync.dma_start(accD_d[:, :cut], accD[:, :cut])
            nc.sync.dma_start(accD_d[:, cut:], accD[:, cut:])

    nc.finalize()
    return nc, n_cols


def _prep_labels(y, Y2):
    """fp8 DoubleRow operands: class c -> (partition c%64, subtile c//64)."""
    y_st = np.zeros((64, 2, B), FP8)
    YT = np.zeros((64, 2, Y2.shape[0]), FP8)
    for j in (0, 1):
        w = min(64, C - 64 * j)
        y_st[:w, j, :] = y.T[64 * j : 64 * j + w].astype(FP8)
        YT[:w, j, :] = Y2.T[64 * j : 64 * j + w].astype(FP8)
    return y_st.reshape(64, 2 * B), YT


def _prep_host(u, y, ind, U, Y):
    """Scatter + device arrays + fp64 base sum."""
    u = np.asarray(u, dtype=np.float32)
    y = np.asarray(y, dtype=np.float32)
    ind = np.asarray(ind).astype(np.int64)
    U2 = np.array(U, dtype=np.float32, copy=True)
    Y2 = np.array(Y, dtype=np.float32, copy=True)
    U2[ind] = u
    Y2[ind] = y

    u64 = u.astype(np.float64)
    U64 = U2.astype(np.float64)
    u_sq64 = (u64 * u64).sum(axis=1)
    U_sq64 = (U64 * U64).sum(axis=1)
    s_raw = (
        N * u_sq64.sum()
        + B * U_sq64.sum()
        - 2.0 * (u64.sum(axis=0) @ U64.sum(axis=0))
    )
    y2d, YT = _prep_labels(y, Y2)
    return u, y, U2, Y2, y2d, YT, s_raw


def _full_numpy_loss(u, y, U2, Y2):
    """Exact fp64 fallback (blocked); only used if detector preconditions
    fail (non-binary labels) -- never on spec inputs."""
    total = 0.0
    U64 = U2.astype(np.float64)
    Y64 = Y2.astype(np.float64)
    U_sq = (U64 * U64).sum(axis=1)
    for b0 in range(0, B, 64):
        ub = u[b0 : b0 + 64].astype(np.float64)
        yb = y[b0 : b0 + 64].astype(np.float64)
        dist = np.maximum(
            (ub * ub).sum(1)[:, None] - 2.0 * (ub @ U64.T) + U_sq[None, :], 0.0)
        mism = (yb @ Y64.T) == 0.0
        total += np.where(mism, 0.5 * np.maximum(M_MARGIN - dist, 0.0),
                          0.5 * dist).sum()
    loss1 = total / (B * N)
    loss2 = ALPHA * np.abs(1.0 - np.sign(u)).mean(dtype=np.float64)
    return np.array(loss1 + loss2, dtype=np.float32)


def _detector_preconditions_ok(y, Y2):
    return bool(((y == 0.0) | (y == 1.0)).all()
                and ((Y2 == 0.0) | (Y2 == 1.0)).all())


def _decode_flags(accD, n_sh):
    """Candidate (batch index, g0, g1) spans from the detector columns.

    ACT cols: sum of relu(0.5 - s), flag > 0.25. TTR cols: clipped
    min over direct + encoded pair, flag < 0.25.
    """
    ops, n_cols = _schedule(n_sh)
    cands = []
    for kind, col, units in ops:
        colv = accD[:, col]
        flagged = colv > 0.25 if kind == "ACT" else colv < 0.5
        for p in np.nonzero(flagged)[0]:
            for bt, g0, g1 in units:
                cands.append((bt * 128 + int(p), g0, min(g1, n_sh)))
    return cands


_PROG_CACHE = {}


def _get_program():
    key = ("v5", N_SH)
    if key not in _PROG_CACHE:
        _PROG_CACHE[key] = _build_program(N_SH)
    return _PROG_CACHE[key]


def kernel(u, y, ind, U, Y):
    u, y, U2, Y2, y2d, YT, s_raw = _prep_host(u, y, ind, U, Y)

    if not _detector_preconditions_ok(y, Y2):
        return _full_numpy_loss(u, y, U2, Y2)

    nc, n_cols = _get_program()
    in_maps = []
    for c in range(N_CORES):
        ns = slice(c * N_SH, (c + 1) * N_SH)
        in_maps.append({
            "y2": y2d,
            "YT": np.ascontiguousarray(
                YT[:, :, ns].reshape(64, 2 * N_SH)),
        })

    res = run_bass_kernel_spmd(nc, in_maps, list(range(N_CORES)))
    results = res.results

    y64 = y.astype(np.float64)
    Y64 = Y2.astype(np.float64)
    corr = 0.0
    seen = set()
    for c in range(N_CORES):
        accD = np.asarray(results[c]["accD"], dtype=np.float64)
        for b, g0, g1 in _decode_flags(accD, N_SH):
            if g0 >= g1:
                continue
            n0, n1 = c * N_SH + g0, c * N_SH + g1
            match = Y64[n0:n1] @ y64[b]
            for z in np.nonzero(match == 0.0)[0]:
                key = (b, n0 + z)
                if key in seen:
                    continue
                seen.add(key)
                d = u[b].astype(np.float64) - U2[n0 + z].astype(np.float64)
                raw = float(d @ d)
                corr += max(M_MARGIN - raw, 0.0) - raw

    total2 = s_raw + corr
    loss1 = 0.5 * total2 / (B * N)
    loss2 = ALPHA * np.abs(1.0 - np.sign(u)).mean(dtype=np.float64)
    return np.array(loss1 + loss2, dtype=np.float32)


# revision 22
# speedup vs baseline: 1.2928x; 1.2928x over previous
"""DSH loss kernel for Trainium2 (8 NeuronCores, Bass/Tile).

Math (reference):
    U[ind] = u; Y[ind] = y
    raw[b,n]  = ||u_b||^2 - 2 u_b.U_n + ||U_n||^2          (>= 0 mathematically)
    dist      = max(raw, 0)
    match[b,n]= y_b . Y_n          (integer >= 0)
    m         = (match == 0)       ("mismatch" mask, statistically ~never 1)
    loss1 = mean( (1-m)*0.5*dist + m*0.5*relu(M - dist) )
    loss2 = ALPHA * mean(|1 - sign(u)|)

Decomposition (v6):
    2*B*N*loss1 = S_raw + sum_{m=1} [ relu(M - raw) - raw ]
      S_raw factorizes (N*sum(u_sq) + B*sum(U_sq) - 2*colsum(u).colsum(U))
      -> exact fp64 on host. Distances never touch the device: the
      correction only needs the LOCATIONS of match==0 pairs, found from
      the labels alone (half the matmul work of computing distances).

Device work per core (shard = 12500 gallery rows, padded to 12544):
    match[b,n] = y_b . Y_n, one bf16 K=128 matmul per 128-row gallery
    tile (binary labels are bf16-exact; fp32 PSUM accumulation gives
    exact integer counts). Measured per-MM cost on this part is ~320ns
    (K=128, free=512) regardless of dtype; fp8 / DoubleRow / smaller K
    are all equal or slower, and out free > 512 is rejected by ISA
    checks, so 98 matmuls is the PE floor. A short warm-up burst of
    y x y dummy matmuls runs during the initial gallery-DMA window.

    Zero-match detector, one probe per pair over 49 super-tiles
    [128, 1024] (2 PSUM banks = 2 gallery tiles x 512 batch):
      - ScalarE: activation Relu(0.5 - s) + accum_out column
        (> 0.25 flags a zero-match); ~1396ns/super incl accumulator
        read.
      - VectorE: tensor_reduce min column (< 0.5 flags); ~1218ns/super.
    Supers are assigned greedily by those measured costs so both
    engines finish together (~32us). Gallery DMA is split across the
    sync + gpsimd rings, small chunks first so tile 0 lands ASAP.
    Flagged (gallery row, batch-all) candidates are re-checked exactly
    on host in fp64 (normally there are none).
"""

import numpy as np
import ml_dtypes

import concourse.bass as bass
import concourse.mybir as mybir
import concourse.tile as tile
from concourse import bacc
from concourse.bass_utils import run_bass_kernel_spmd

# Problem constants (hardcoded per harness contract)
B = 512
BIT = 64
C = 100
N = 100000
N_CORES = 8
N_SH = N // N_CORES          # 12500
M_MARGIN = 2.0 * BIT         # 128.0
ALPHA = 0.1
P_TILE = 128                 # gallery rows per tile
N_WARMUP = 10                # PE warm-up dummy matmuls

BF16 = ml_dtypes.bfloat16

# measured per-instruction EW costs (ns) on [128,1024] supers
_COST_ACT = 1396.0
_COST_DVE = 1218.0


def _layout(n_sh: int):
    n_pad = ((n_sh + 2 * P_TILE - 1) // (2 * P_TILE)) * (2 * P_TILE)
    n_ps = n_pad // (2 * P_TILE)      # [128,1024] super-tiles == accD cols
    return n_pad, n_ps, n_ps


def _schedule(n_sh: int):
    """Greedy engine assignment per super-tile, balancing measured
    per-instruction costs. Returns (kinds, n_cols): kinds[pi] in
    {'ACT','DVE'}; accD col == pi covers gallery tiles {2pi, 2pi+1}."""
    n_pad, n_ps, n_cols = _layout(n_sh)
    kinds = []
    t_act = t_dve = 0.0
    for pi in range(n_ps):
        if t_act + _COST_ACT <= t_dve + _COST_DVE:
            kinds.append("ACT")
            t_act += _COST_ACT
        else:
            kinds.append("DVE")
            t_dve += _COST_DVE
    return kinds, n_cols


def _build_program(n_sh: int):
    fp32 = mybir.dt.float32
    bf16 = mybir.dt.bfloat16
    nc = bacc.Bacc("TRN2", target_bir_lowering=False)

    n_pad, n_ps, n_cols = _layout(n_sh)
    kinds, _ = _schedule(n_sh)
    amin = mybir.AluOpType.min

    ypT_d = nc.declare_dram_parameter("ypT", [128, B], bf16, isOutput=False)
    YT_d = nc.declare_dram_parameter("YT", [128, n_sh], bf16, isOutput=False)
    accD_d = nc.declare_dram_parameter("accD", [128, n_cols], fp32, isOutput=True)

    with tile.TileContext(nc) as tc:
        with (
            tc.tile_pool(name="resident", bufs=1) as resident,
            tc.tile_pool(name="scr", bufs=2) as scrp,
            tc.tile_pool(name="psum", bufs=4, space="PSUM") as psump,
        ):
            yp_sb = resident.tile([128, B], bf16, tag="yp")
            YT_sb = resident.tile([128, n_pad], bf16, tag="YT")
            accD = resident.tile([128, n_cols], fp32, tag="accD")
            bias_h = resident.tile([128, 1], fp32, tag="biash")

            # moving operand on the otherwise-idle scalar ring so it
            # lands first; gallery stream on sync+gpsimd, small first
            nc.scalar.dma_start(yp_sb[:], ypT_d[:])
            half = n_sh // 2
            for base, end, q in ((0, half, nc.sync), (half, n_sh, nc.gpsimd)):
                s = base
                widths = [128, 128, 256, 512, 1024, 2048] + [4096] * 8
                for w in widths:
                    if s >= end:
                        break
                    w = min(w, end - s)
                    q.dma_start(YT_sb[:, s : s + w], YT_d[:, s : s + w])
                    s += w
                if s < end:
                    q.dma_start(YT_sb[:, s:end], YT_d[:, s:end])
            if n_pad > n_sh:
                nc.vector.memset(YT_sb[:, n_sh:], 1.0)
            nc.vector.memset(bias_h[:], 0.5)
            nc.vector.memset(accD[:], 1.0)

            for pi in range(n_ps):
                x = psump.tile([P_TILE, 1024], fp32, tag="x")
                if pi == 0:
                    # PE warm-up on the moving operand during DMA wait
                    for _ in range(N_WARMUP):
                        nc.tensor.matmul(
                            x[:, :512], lhsT=yp_sb[:, :128], rhs=yp_sb[:],
                            start=True, stop=True,
                        )
                for h in (0, 1):
                    t = 2 * pi + h
                    ns = slice(t * P_TILE, (t + 1) * P_TILE)
                    nc.tensor.matmul(
                        x[:, h * 512 : (h + 1) * 512],
                        lhsT=YT_sb[:, ns], rhs=yp_sb[:],
                        start=True, stop=True,
                    )
                col = accD[:, pi : pi + 1]
                if kinds[pi] == "ACT":
                    scrA = scrp.tile([P_TILE, 1024], bf16, tag="scrA")
                    nc.scalar.activation(
                        scrA[:], x[:],
                        mybir.ActivationFunctionType.Relu,
                        bias=bias_h[:], scale=-1.0,
                        accum_out=col,
                    )
                else:
                    nc.vector.tensor_reduce(
                        col, x[:], mybir.AxisListType.X, amin,
                    )

            cut = max(0, n_cols - 5)
            if cut:
                nc.sync.dma_start(accD_d[:, :cut], accD[:, :cut])
            nc.sync.dma_start(accD_d[:, cut:], accD[:, cut:])

    nc.finalize()
    return nc, n_cols


def _prep_host(u, y, ind, U, Y):
    """Scatter + device arrays (bf16) + fp64 base sum."""
    u = np.asarray(u, dtype=np.float32)
    y = np.asarray(y, dtype=np.float32)
    ind = np.asarray(ind).astype(np.int64)
    U2 = np.array(U, dtype=np.float32, copy=True)
    Y2 = np.array(Y, dtype=np.float32, copy=True)
    U2[ind] = u
    Y2[ind] = y

    u64 = u.astype(np.float64)
    U64 = U2.astype(np.float64)
    u_sq64 = (u64 * u64).sum(axis=1)
    U_sq64 = (U64 * U64).sum(axis=1)
    s_raw = (
        N * u_sq64.sum()
        + B * U_sq64.sum()
        - 2.0 * (u64.sum(axis=0) @ U64.sum(axis=0))
    )

    ypT = np.zeros((128, B), dtype=BF16)
    ypT[:C] = y.T.astype(BF16)
    YT = np.zeros((128, N), dtype=BF16)
    YT[:C] = Y2.T.astype(BF16)

    return u, y, U2, Y2, ypT, YT, s_raw


def _full_numpy_loss(u, y, U2, Y2):
    """Exact fp64 fallback (blocked); only used if detector preconditions
    fail (non-binary labels) -- never on spec inputs."""
    total = 0.0
    U64 = U2.astype(np.float64)
    Y64 = Y2.astype(np.float64)
    U_sq = (U64 * U64).sum(axis=1)
    for b0 in range(0, B, 64):
        ub = u[b0 : b0 + 64].astype(np.float64)
        yb = y[b0 : b0 + 64].astype(np.float64)
        dist = np.maximum(
            (ub * ub).sum(1)[:, None] - 2.0 * (ub @ U64.T) + U_sq[None, :], 0.0)
        mism = (yb @ Y64.T) == 0.0
        total += np.where(mism, 0.5 * np.maximum(M_MARGIN - dist, 0.0),
                          0.5 * dist).sum()
    loss1 = total / (B * N)
    loss2 = ALPHA * np.abs(1.0 - np.sign(u)).mean(dtype=np.float64)
    return np.array(loss1 + loss2, dtype=np.float32)


def _detector_preconditions_ok(y, Y2):
    return bool(((y == 0.0) | (y == 1.0)).all()
                and ((Y2 == 0.0) | (Y2 == 1.0)).all())


def _decode_flags(accD, n_sh):
    """Candidate local gallery rows. Col pi covers gallery tiles
    {2pi, 2pi+1}; ACT cols flag > 0.25, DVE cols flag < 0.5."""
    kinds, n_cols = _schedule(n_sh)
    cand = set()
    for pi, kind in enumerate(kinds):
        col = accD[:, pi]
        ps = np.nonzero(col > 0.25 if kind == "ACT" else col < 0.5)[0]
        for p in ps:
            for j in range(2):
                cand.add((2 * pi + j) * P_TILE + int(p))
    return sorted(n for n in cand if n < n_sh)


_PROG_CACHE = {}


def _get_program():
    key = ("v6", N_SH)
    if key not in _PROG_CACHE:
        _PROG_CACHE[key] = _build_program(N_SH)
    return _PROG_CACHE[key]


def kernel(u, y, ind, U, Y):
    u, y, U2, Y2, ypT, YT, s_raw = _prep_host(u, y, ind, U, Y)

    if not _detector_preconditions_ok(y, Y2):
        return _full_numpy_loss(u, y, U2, Y2)

    nc, n_cols = _get_program()
    in_maps = []
    for c in range(N_CORES):
        ns = slice(c * N_SH, (c + 1) * N_SH)
        in_maps.append({
            "ypT": ypT,
            "YT": np.ascontiguousarray(YT[:, ns]),
        })

    res = run_bass_kernel_spmd(nc, in_maps, list(range(N_CORES)))
    results = res.results

    y64 = y.astype(np.float64)
    corr = 0.0
    for c in range(N_CORES):
        accD = np.asarray(results[c]["accD"], dtype=np.float64)
        for n_loc in _decode_flags(accD, N_SH):
            n_glob = c * N_SH + n_loc
            match = y64 @ Y2[n_glob].astype(np.float64)
            for b in np.nonzero(match == 0.0)[0]:
                d = u[b].astype(np.float64) - U2[n_glob].astype(np.float64)
                raw = float(d @ d)
                corr += max(M_MARGIN - raw, 0.0) - raw

    total2 = s_raw + corr
    loss1 = 0.5 * total2 / (B * N)
    loss2 = ALPHA * np.abs(1.0 - np.sign(u)).mean(dtype=np.float64)
    return np.array(loss1 + loss2, dtype=np.float32)
